# revision 13
# baseline (speedup 1.0000x reference)
"""ConvLIF-WTA Trainium2 kernel (raw Bass, explicit semaphores).

Reference computation:
  u = causal_conv1d(x[B,1,T], W[K,1,ks])          -> [B,K,T]
  LIF scan over t with winner-take-all:
    v = ALPHA*v + BETA*u_t
    s = onehot(argmax_k v) * (v_max >= THETA)
    v = v - THETA*s
  output spikes [B,K,T] f32.

Per-core pipeline (8 cores, batch-parallel, 32 batch rows per core):
  SP   : sliding-window DMA xp->Xwin[16,(b,t)], spike chunk stores
  PE   : conv matmuls (BETA*W)^T[16,64] @ Xwin -> psum u[k,(b,t)]
  ACT  : psum -> SBUF copy (DMA cannot read PSUM)
  POOL : DMA bounce through internal DRAM: (k,(b,t)) -> (b,(k,t)) relayout
  DVE  : sequential WTA scan, 4 ops per step on [32,64] tiles:
           1. v = (v * ALPHA) + u_t             (scalar_tensor_tensor)
           2. c = reduce_max over [32,65] view  (col 65 preset to THETA,
                                                 so c = max(max_k v, THETA))
           3. s = (v >= c)                      (tensor_scalar per-part ptr)
           4. v = (s * -THETA) + v              (scalar_tensor_tensor)
         If max_k v >= THETA only the argmax hits equality; otherwise
         c==THETA and nothing spikes.  Matches reference up to
         measure-zero ties.

Raw Bass because: this walrus encodes at most ONE fused sync-wait per
instruction; Tile attaches multi-sem on_wait lists and the compile dies
with "Too many sync wait commands".  Explicit wait_ge instructions have
no such limit.
"""

import dataclasses
import numpy as np
from contextlib import ExitStack

import concourse.bass as bass
import concourse.mybir as mybir
from concourse.bass_utils import run_bass_kernel_spmd

# Problem constants (hardcoded per contract)
B_FULL = 256
T = 4096
K = 64
KS = 16
PAD = KS - 1
N_CORES = 8
B = B_FULL // N_CORES  # 32

TAU = 10.0
THETA = 0.5
ALPHA = float(np.exp(-1.0 / TAU))
BETA = 1.0 - ALPHA

TC = 64
NCHUNK = T // TC
FP32 = mybir.dt.float32

_cache = {}


def _build():
    nc = bass.Bass()
    xp_h = nc.declare_dram_parameter("xp", [B, PAD + T], FP32, isOutput=False)
    w_h = nc.declare_dram_parameter("W", [K, KS], FP32, isOutput=False)
    out_h = nc.declare_dram_parameter("out", [B, K, T], FP32, isOutput=True)
    u_dram = nc.dram_tensor("u_dram", [B, K, T], FP32)

    es = ExitStack()
    # SBUF / PSUM allocations (live for the whole program)
    wt_raw = es.enter_context(nc.sbuf_tensor("wt_raw", [KS, K], FP32))
    wt = es.enter_context(nc.sbuf_tensor("wt", [KS, K], FP32))
    v = es.enter_context(nc.sbuf_tensor("v", [B, K + 1], FP32))
    cmax = es.enter_context(nc.sbuf_tensor("cmax", [B, 1], FP32))
    xwin = [
        es.enter_context(nc.sbuf_tensor(f"xwin{i}", [KS, B * TC], FP32))
        for i in range(2)
    ]
    cu = [
        es.enter_context(nc.sbuf_tensor(f"cu{i}", [K, B * TC], FP32))
        for i in range(2)
    ]
    u_sb = [
        es.enter_context(nc.sbuf_tensor(f"u_sb{i}", [B, K * TC], FP32))
        for i in range(3)
    ]
    s_sb = [
        es.enter_context(nc.sbuf_tensor(f"s_sb{i}", [B, K * TC], FP32))
        for i in range(3)
    ]
    pu = [
        es.enter_context(nc.psum_tensor(f"pu{i}", [K, B * TC], FP32))
        for i in range(2)
    ]

    sem_prep_dma = es.enter_context(nc.semaphore("prep_dma"))
    sem_prep = es.enter_context(nc.semaphore("prep"))
    sem_xw = es.enter_context(nc.semaphore("xw"))
    sem_mm = es.enter_context(nc.semaphore("mm"))
    sem_cu = es.enter_context(nc.semaphore("cuc"))
    sem_st = es.enter_context(nc.semaphore("st"))
    sem_ld = es.enter_context(nc.semaphore("ld"))
    sem_scan = es.enter_context(nc.semaphore("scan"))
    sem_out = es.enter_context(nc.semaphore("outs"))

    xpad_row = PAD + T
    NBLK = (B * TC) // 512  # matmuls per chunk

    with nc.Block() as block:

        @block.sync
        def _(sp):
            # prep: W^T load
            with nc.allow_non_contiguous_dma(reason="4KB one-time W transpose"):
                sp.dma_start(
                    out=wt_raw[:, :], in_=w_h[:, :].rearrange("k i -> i k")
                ).then_inc(sem_prep_dma, 16)
            for c in range(NCHUNK):
                t0 = c * TC
                # xwin load (WAR: matmuls of chunk c-2 done with slot c%2)
                if c >= 2:
                    sp.wait_ge(sem_mm, c - 1)
                src = dataclasses.replace(
                    xp_h[:, :],
                    ap=[[1, KS], [xpad_row, B], [1, TC]],
                    offset=t0,
                )
                sp.dma_start(
                    out=xwin[c % 2][:, :].rearrange("p (b t) -> p b t", b=B),
                    in_=src,
                ).then_inc(sem_xw, 16)
                # spike store of chunk c-1
                if c >= 1:
                    sp.wait_ge(sem_scan, c)
                    sv = s_sb[(c - 1) % 3][:, :].rearrange("b (k t) -> b k t", k=K)
                    sp.dma_start(
                        out=out_h[:, :, (c - 1) * TC : c * TC], in_=sv
                    ).then_inc(sem_out, 16)
            sp.wait_ge(sem_scan, NCHUNK)
            sv = s_sb[(NCHUNK - 1) % 3][:, :].rearrange("b (k t) -> b k t", k=K)
            sp.dma_start(
                out=out_h[:, :, T - TC : T], in_=sv
            ).then_inc(sem_out, 16)

        @block.tensor
        def _(pe):
            pe.wait_ge(sem_prep, 1)
            for c in range(NCHUNK):
                pe.wait_ge(sem_xw, 16 * (c + 1))
                if c >= 2:
                    pe.wait_ge(sem_cu, c - 1)  # psum slot WAR: ACT copy c-2 done
                for j in range(NBLK):
                    pe.matmul(
                        pu[c % 2][:, j * 512 : (j + 1) * 512],
                        wt[:, :],
                        xwin[c % 2][:, j * 512 : (j + 1) * 512],
                        start=True,
                        stop=True,
                    )
                pe.drain().then_inc(sem_mm, 1)

        @block.scalar
        def _(act):
            for c in range(NCHUNK):
                act.wait_ge(sem_mm, c + 1)
                if c >= 2:
                    act.wait_ge(sem_st, 16 * (c - 1))  # cu slot WAR: store c-2
                act.copy(cu[c % 2][:, :], pu[c % 2][:, :])
                act.drain().then_inc(sem_cu, 1)

        @block.gpsimd
        def _(pool):
            for c in range(NCHUNK):
                t0 = c * TC
                pool.wait_ge(sem_cu, c + 1)
                dst = dataclasses.replace(
                    u_dram[:, :, :],
                    ap=[[T, K], [K * T, B], [1, TC]],
                    offset=t0,
                )
                pool.dma_start(
                    out=dst,
                    in_=cu[c % 2][:, :].rearrange("k (b t) -> k b t", b=B),
                ).then_inc(sem_st, 16)
                pool.wait_ge(sem_st, 16 * (c + 1))
                if c >= 3:
                    pool.wait_ge(sem_scan, c - 2)  # u_sb slot WAR: scan c-3 done
                pool.dma_start(
                    out=u_sb[c % 3][:, :].rearrange("b (k t) -> b k t", k=K),
                    in_=u_dram[:, :, t0 : t0 + TC],
                ).then_inc(sem_ld, 16)

        @block.vector
        def _(dve):
            # prep
            dve.memset(v[:, :K], 0.0)
            dve.memset(v[:, K : K + 1], THETA)
            dve.wait_ge(sem_prep_dma, 16)
            dve.tensor_scalar_mul(wt[:, :], wt_raw[:, :], BETA)
            dve.drain().then_inc(sem_prep, 1)
            for c in range(NCHUNK):
                dve.wait_ge(sem_ld, 16 * (c + 1))
                if c >= 3:
                    dve.wait_ge(sem_out, 16 * (c - 2))  # s_sb slot WAR: store c-3
                u_v = u_sb[c % 3][:, :].rearrange("b (k t) -> b k t", k=K)
                s_v = s_sb[c % 3][:, :].rearrange("b (k t) -> b k t", k=K)
                for t in range(TC):
                    dve.scalar_tensor_tensor(
                        v[:, :K], v[:, :K], ALPHA, u_v[:, :, t],
                        op0=mybir.AluOpType.mult, op1=mybir.AluOpType.add,
                    )
                    dve.drain()
                    dve.tensor_reduce(
                        cmax[:, :], v[:, :], axis=mybir.AxisListType.X,
                        op=mybir.AluOpType.max,
                    )
                    dve.drain()
                    dve.tensor_scalar(
                        s_v[:, :, t], v[:, :K], cmax[:, :], None,
                        op0=mybir.AluOpType.is_ge,
                    )
                    dve.drain()
                    ins = dve.scalar_tensor_tensor(
                        v[:, :K], s_v[:, :, t], -THETA, v[:, :K],
                        op0=mybir.AluOpType.mult, op1=mybir.AluOpType.add,
                    )
                    drn = dve.drain()
                    if t == TC - 1:
                        drn.then_inc(sem_scan, 1)

    es.close()
    return nc


def kernel(x: np.ndarray, W: np.ndarray) -> np.ndarray:
    if "nc" not in _cache:
        _cache["nc"] = _build()
    nc = _cache["nc"]

    x2 = np.ascontiguousarray(x.reshape(B_FULL, T).astype(np.float32))
    xp = np.pad(x2, ((0, 0), (PAD, 0)))
    w2 = np.ascontiguousarray(W.reshape(K, KS).astype(np.float32))
    in_maps = [
        {"xp": xp[i * B : (i + 1) * B], "W": w2} for i in range(N_CORES)
    ]
    res = run_bass_kernel_spmd(nc, in_maps, list(range(N_CORES)))
    outs = [res.results[i]["out"].reshape(B, K, T) for i in range(N_CORES)]
    return np.concatenate(outs, axis=0).astype(np.float32)
